# revision 12
# baseline (speedup 1.0000x reference)
"""Trainium2 Bass kernel for nn_GCNClusterGAT (2-layer GAT + soft k-means).

Strategy (8 NeuronCores, SPMD):
  - Nodes sharded in 8 contiguous blocks of 2500; each core owns the edges
    whose *destination* lies in its block (dst-sharding -> segment softmax
    and scatter-sum are core-local).
  - Host preprocessing (indices only): append self-loops, bucket edges by
    owning core, sort by local dst, split into 128-dst windows and 128-edge
    chunks, build per-chunk one-hot (edge -> dst-in-window) matrices as fp8
    plus their transposes, and wrap gather indices for the SWDGE gather.
  - GAT projections are data-parallel; attention coefficient projections are
    folded into the weight matrix (W_ext = [W^T | W^T a_src | W^T a_dst]).
  - Per-node rows [h | 1 | alpha_src] are AllGathered, then each core
    dma_gathers the rows of its edges' sources and does the segment-softmax
    weighted sum as PSUM-accumulated one-hot matmuls (exp folded into the
    one-hot via per-partition scalar multiply; the "1" columns produce the
    softmax denominators for free).
  - Soft k-means: node-sharded; the [K,65] partial sums ([r^T dn | r^T 1])
    are AllGathered and reduced on every core each iteration.
"""

import numpy as np

import concourse.bacc as bacc
import concourse.bass as bass
import concourse.mybir as mybir
import concourse.tile as tile
from concourse.bass_utils import run_bass_kernel_spmd
from concourse.masks import make_identity

F32 = mybir.dt.float32
FP8 = mybir.dt.float8e4
I16 = mybir.dt.int16
AF = mybir.ActivationFunctionType
OP = mybir.AluOpType
AX = mybir.AxisListType

N = 20000
NCORES = 8
NPC = N // NCORES          # 2500 nodes per core
NB = 20                    # 128-node blocks per core (last block has 68)
LASTM = NPC - (NB - 1) * 128   # 68
NFEAT = 512
NHID = 128
HEADS = 2
H1 = HEADS * NHID          # 256
NOUT = 64
K = 100
ALPHA = 0.2
TEMP = 5.0
ROW1 = 320                 # gather row: [h0(128) 1 h1(128) 1 as0 as1 pad]
ROW2 = 128                 # gather row: [h2(64) 1 as2 pad]
G = 8                      # chunks per gather/one-hot group

_cache = {}
TRACE = False          # set by test harness to capture an NTFF profile
LAST_RESULT = None     # BassKernelResults of the most recent run


def _host_prep(edge_index):
    """Index-only preprocessing: per-core dst-sorted chunked edge structure."""
    src = np.concatenate([edge_index[0], np.arange(N, dtype=np.int64)])
    dst = np.concatenate([edge_index[1], np.arange(N, dtype=np.int64)])
    core = dst // NPC
    per_core = []
    cmax = 0
    for c in range(NCORES):
        m = core == c
        s, d = src[m], dst[m] - c * NPC
        order = np.argsort(d, kind="stable")
        s, d = s[order], d[order]
        win = d // 128
        cnt = np.bincount(win, minlength=NB)
        cmax = max(cmax, int(np.ceil(cnt.max() / 128)))
        per_core.append((s, d, cnt))
    if cmax % 2:
        cmax += 1              # keep 20*cmax divisible by 8
    nch = NB * cmax
    epc = nch * 128
    idx_all = np.zeros((NCORES, epc), np.int16)
    col_all = np.zeros((NCORES, epc), np.int64)       # dst col within window
    valid_all = np.zeros((NCORES, epc), bool)
    for c in range(NCORES):
        s, d, cnt = per_core[c]
        off = 0
        for w in range(NB):
            n = int(cnt[w])
            base = w * cmax * 128
            idx_all[c, base:base + n] = s[off:off + n].astype(np.int16)
            col_all[c, base:base + n] = d[off:off + n] - w * 128
            valid_all[c, base:base + n] = True
            off += n
    # one-hot fp8 arrays, grouped by G chunks for wide DMA:
    # oh[g, p, k*128+col]  = 1 if edge p of chunk g*G+k has dst col `col`
    # ohT[g, col, k*128+p] = same, transposed per chunk
    ng = nch // G
    fp8np = mybir.dt.np(FP8)
    oh = np.zeros((NCORES, ng, 128, G * 128), fp8np)
    ohT = np.zeros((NCORES, ng, 128, G * 128), fp8np)
    one = np.array(1.0, dtype=fp8np)
    cc = np.arange(epc) // 128          # chunk of each edge slot
    pp = np.arange(epc) % 128           # slot within chunk
    for c in range(NCORES):
        v = valid_all[c]
        gg, kk = cc[v] // G, cc[v] % G
        oh[c, gg, pp[v], kk * 128 + col_all[c, v]] = one
        ohT[c, gg, col_all[c, v], kk * 128 + pp[v]] = one
    # wrap indices for dma_gather: idx i -> [i%16 (replicated x8), i//16]
    idx16 = np.zeros((NCORES, 128, epc // 16), np.int16)
    for c in range(NCORES):
        idx16[c] = np.tile(idx_all[c].reshape(-1, 16).T, (8, 1))
    return cmax, idx16, oh, ohT


def _build(cmax, niter):
    nch = NB * cmax
    epc = nch * 128
    ng = nch // G
    nc = bacc.Bacc("TRN2", target_bir_lowering=False, debug=False,
                   num_devices=NCORES)
    dt = nc.dram_tensor
    # ---- external inputs (per core) ----
    xT_in = dt("xT_in", [NFEAT, NPC], F32, kind="ExternalInput").ap()
    w1_in = dt("w1_in", [NFEAT, H1 + 4], F32, kind="ExternalInput").ap()
    w2_in = dt("w2_in", [H1, NOUT + 2], F32, kind="ExternalInput").ap()
    b1_in = dt("b1_in", [128, H1], F32, kind="ExternalInput").ap()
    b2_in = dt("b2_in", [128, NOUT], F32, kind="ExternalInput").ap()
    mu0_in = dt("mu0_in", [NOUT, K], F32, kind="ExternalInput").ap()
    idx_in = dt("idx_in", [128, epc // 16], I16, kind="ExternalInput").ap()
    oh_in = dt("oh_in", [ng, 128, G * 128], FP8, kind="ExternalInput").ap()
    ohT_in = dt("ohT_in", [ng, 128, G * 128], FP8, kind="ExternalInput").ap()
    # ---- external outputs ----
    emb_out = dt("emb_out", [NPC, NOUT], F32, kind="ExternalOutput").ap()
    r_out = dt("r_out", [NPC, K], F32, kind="ExternalOutput").ap()
    dist_out = dt("dist_out", [NPC, K], F32, kind="ExternalOutput").ap()
    mu_out = dt("mu_out", [K, NOUT], F32, kind="ExternalOutput").ap()
    # ---- internal DRAM ----
    hx1_loc = dt("hx1_loc", [NPC, ROW1], F32).ap()
    hx1_full = dt("hx1_full", [N * ROW1], F32, addr_space="Shared").ap()
    hx2_loc = dt("hx2_loc", [NPC, ROW2], F32).ap()
    hx2_full = dt("hx2_full", [N * ROW2], F32, addr_space="Shared").ap()
    part_loc = dt("part_loc", [K, NOUT + 1], F32).ap()
    part_full = dt("part_full", [NCORES * K * (NOUT + 1)], F32,
                   addr_space="Shared").ap()

    rg = [list(range(NCORES))]

    def blk(i):
        return 128 if i < NB - 1 else LASTM

    with tile.TileContext(nc, num_cores=NCORES) as tc:
        with tc.tile_pool(name="persist", bufs=1) as pp, \
             tc.tile_pool(name="work", bufs=2) as wp, \
             tc.tile_pool(name="small", bufs=4) as sp:
            ident = pp.tile([128, 128], F32, tag="ident")
            make_identity(nc, ident[:])
            w1_sb = pp.tile([128, 4, H1 + 4], F32, tag="w1")
            nc.sync.dma_start(w1_sb[:], w1_in.rearrange("(c p) f -> p c f", p=128))
            w2_sb = pp.tile([128, 2, NOUT + 2], F32, tag="w2")
            nc.sync.dma_start(w2_sb[:], w2_in.rearrange("(c p) f -> p c f", p=128))
            b1_sb = pp.tile([128, H1], F32, tag="b1")
            nc.sync.dma_start(b1_sb[:], b1_in[:])
            b2_sb = pp.tile([128, NOUT], F32, tag="b2")
            nc.sync.dma_start(b2_sb[:], b2_in[:])
            idx_sb = pp.tile([128, epc // 16], I16, tag="idx")
            nc.sync.dma_start(idx_sb[:], idx_in[:])
            ad1_sb = pp.tile([128, NB * 2], F32, tag="ad1")
            ad2_sb = pp.tile([128, NB], F32, tag="ad2")
            agg1_sb = pp.tile([128, NB * H1], F32, tag="agg1")
            dnx_sb = pp.tile([128, NB, NOUT + 1], F32, tag="dnx")
            nc.vector.memset(dnx_sb[:], 1.0)

            # ================= phase 1: x @ W1_ext =================
            with tc.tile_pool(name="xT", bufs=1) as xp, \
                 tc.tile_pool(name="pj", bufs=2, space="PSUM") as pjp:
                xT_sb = xp.tile([128, 4, NPC], F32, tag="xT")
                nc.sync.dma_start(xT_sb[:], xT_in.rearrange("(c p) n -> p c n", p=128))
                for nb in range(NB):
                    m = blk(nb)
                    ph = pjp.tile([128, H1 + 4], F32, space="PSUM", tag="ph")
                    for fc in range(4):
                        nc.tensor.matmul(
                            ph[:m, :], lhsT=xT_sb[:, fc, nb * 128:nb * 128 + m],
                            rhs=w1_sb[:, fc, :], start=(fc == 0), stop=(fc == 3))
                    hx = wp.tile([128, ROW1], F32, tag="hx1")
                    nc.vector.tensor_copy(hx[:m, 0:128], ph[:m, 0:128])
                    nc.vector.tensor_copy(hx[:m, 129:257], ph[:m, 128:256])
                    nc.vector.tensor_copy(hx[:m, 258:260], ph[:m, 256:258])
                    nc.vector.memset(hx[:m, 128:129], 1.0)
                    nc.vector.memset(hx[:m, 257:258], 1.0)
                    nc.vector.memset(hx[:m, 260:ROW1], 0.0)
                    nc.vector.tensor_copy(ad1_sb[:m, nb * 2:nb * 2 + 2],
                                          ph[:m, 258:260])
                    nc.sync.dma_start(hx1_loc[nb * 128:nb * 128 + m, :], hx[:m, :])

            nc.gpsimd.collective_compute(
                "AllGather", OP.bypass, replica_groups=rg,
                ins=[hx1_loc.rearrange("n f -> (n f)")], outs=[hx1_full[:]])

            # ================= shared edge-aggregation phase =================
            def edge_phase(hx_view, row, nhead, hw, ad_sb, epilogue):
                # hw = per-head rhs width (feat + 1); alpha_src at col nhead*hw
                as_off = nhead * hw
                acc_w = {}
                with tc.tile_pool(name="eg", bufs=3) as gp, \
                     tc.tile_pool(name="eo", bufs=3) as op_, \
                     tc.tile_pool(name="es", bufs=4) as scp, \
                     tc.tile_pool(name="eps", bufs=2, space="PSUM") as accp, \
                     tc.tile_pool(name="adps", bufs=2, space="PSUM") as adp:
                    for g in range(ng):
                        gath = gp.tile([128, G, row], F32, tag="gath")
                        nc.gpsimd.dma_gather(
                            gath[:], hx_view[:], idx_sb[:, g * 64:(g + 1) * 64],
                            G * 128, G * 128, row)
                        ohf = op_.tile([128, G * 128], F32, tag="ohf")
                        nc.gpsimd.dma_start(ohf[:], oh_in[g, :, :])
                        ohTf = op_.tile([128, G * 128], F32, tag="ohTf")
                        nc.gpsimd.dma_start(ohTf[:], ohT_in[g, :, :])
                        adps = adp.tile([128, G * nhead], F32, space="PSUM",
                                        tag="adps")
                        for k in range(G):
                            w = (g * G + k) // cmax
                            nc.tensor.matmul(
                                adps[:, k * nhead:(k + 1) * nhead],
                                lhsT=ohTf[:, k * 128:(k + 1) * 128],
                                rhs=ad_sb[:, w * nhead:(w + 1) * nhead],
                                start=True, stop=True, skip_group_check=True)
                        et = scp.tile([128, G, nhead], F32, tag="et")
                        nc.vector.tensor_tensor(
                            et[:], gath[:, :, as_off:as_off + nhead],
                            adps[:].rearrange("p (g h) -> p g h", h=nhead),
                            op=OP.add)
                        lr = scp.tile([128, G, nhead], F32, tag="lr")
                        nc.vector.tensor_scalar_mul(lr[:], et[:], ALPHA)
                        nc.vector.tensor_tensor(lr[:], lr[:], et[:], op=OP.max)
                        ex = scp.tile([128, G, nhead], F32, tag="ex")
                        nc.scalar.activation(ex[:], lr[:], AF.Exp)
                        for k in range(G):
                            w, j = divmod(g * G + k, cmax)
                            if j == 0:
                                acc_w[w] = [
                                    accp.tile([128, hw], F32, name=f"acc{h}",
                                              space="PSUM", tag=f"acc{h}")
                                    for h in range(nhead)]
                            for h in range(nhead):
                                ohx = scp.tile([128, 128], F32, tag=f"ohx{h}")
                                nc.vector.tensor_scalar_mul(
                                    ohx[:], ohf[:, k * 128:(k + 1) * 128],
                                    ex[:, k, h:h + 1])
                                nc.tensor.matmul(
                                    acc_w[w][h][:],
                                    lhsT=ohx[:],
                                    rhs=gath[:, k, h * hw:(h + 1) * hw],
                                    start=(j == 0), stop=(j == cmax - 1),
                                    skip_group_check=True)
                            if j == cmax - 1:
                                epilogue(w, acc_w.pop(w))

            def epi1(w, acc):
                m = blk(w)
                rs = sp.tile([128, 2], F32, tag="rs1")
                nc.vector.reciprocal(rs[:m, 0:1], acc[0][:m, 128:129])
                nc.vector.reciprocal(rs[:m, 1:2], acc[1][:m, 128:129])
                t0 = sp.tile([128, H1], F32, tag="t0")
                nc.vector.tensor_scalar_mul(t0[:m, 0:128], acc[0][:m, 0:128],
                                            rs[:m, 0:1])
                nc.vector.tensor_scalar_mul(t0[:m, 128:256], acc[1][:m, 0:128],
                                            rs[:m, 1:2])
                t1 = sp.tile([128, H1], F32, tag="t1")
                nc.vector.tensor_tensor(t1[:m], t0[:m], b1_sb[:m], op=OP.add)
                q = sp.tile([128, H1], F32, tag="q")
                nc.vector.tensor_scalar_min(q[:m], t1[:m], 0.0)
                q2 = sp.tile([128, H1], F32, tag="q2")
                nc.scalar.activation(q2[:m], q[:m], AF.Exp)
                nc.vector.tensor_scalar_add(q2[:m], q2[:m], -1.0)
                nc.vector.tensor_tensor(agg1_sb[:m, w * H1:(w + 1) * H1],
                                        t1[:m], q2[:m], op=OP.max)

            edge_phase(hx1_full.rearrange("(n f) -> n f", f=ROW1), ROW1,
                       HEADS, NHID + 1, ad1_sb, epi1)

            # ============ phase 3: transpose agg1, project layer 2 ============
            with tc.tile_pool(name="a1T", bufs=1) as a1p, \
                 tc.tile_pool(name="tp", bufs=2, space="PSUM") as tpp, \
                 tc.tile_pool(name="pj2", bufs=2, space="PSUM") as pj2:
                agg1T = a1p.tile([128, 2, NPC], F32, tag="agg1T")
                for nb in range(NB):
                    m = blk(nb)
                    for fc in range(2):
                        tps = tpp.tile([128, 128], F32, space="PSUM", tag="tps")
                        nc.tensor.transpose(
                            tps[:128, :m],
                            agg1_sb[:m, nb * H1 + fc * 128:nb * H1 + (fc + 1) * 128],
                            ident[:m, :m])
                        nc.vector.tensor_copy(
                            agg1T[:, fc, nb * 128:nb * 128 + m], tps[:128, :m])
                for nb in range(NB):
                    m = blk(nb)
                    ph2 = pj2.tile([128, NOUT + 2], F32, space="PSUM", tag="ph2")
                    for fc in range(2):
                        nc.tensor.matmul(
                            ph2[:m, :], lhsT=agg1T[:, fc, nb * 128:nb * 128 + m],
                            rhs=w2_sb[:, fc, :], start=(fc == 0), stop=(fc == 1))
                    h2x = wp.tile([128, ROW2], F32, tag="h2x")
                    nc.vector.tensor_copy(h2x[:m, 0:NOUT], ph2[:m, 0:NOUT])
                    nc.vector.memset(h2x[:m, NOUT:NOUT + 1], 1.0)
                    nc.vector.tensor_copy(h2x[:m, NOUT + 1:NOUT + 2],
                                          ph2[:m, NOUT:NOUT + 1])
                    nc.vector.memset(h2x[:m, NOUT + 2:ROW2], 0.0)
                    nc.vector.tensor_copy(ad2_sb[:m, nb:nb + 1],
                                          ph2[:m, NOUT + 1:NOUT + 2])
                    nc.sync.dma_start(hx2_loc[nb * 128:nb * 128 + m, :], h2x[:m, :])

            nc.gpsimd.collective_compute(
                "AllGather", OP.bypass, replica_groups=rg,
                ins=[hx2_loc.rearrange("n f -> (n f)")], outs=[hx2_full[:]])

            # ================= phase 4: L2 edge aggregation =================
            def epi2(w, acc):
                acc = acc[0]
                m = blk(w)
                rs = sp.tile([128, 1], F32, tag="rs2")
                nc.vector.reciprocal(rs[:m], acc[:m, NOUT:NOUT + 1])
                emb = sp.tile([128, NOUT], F32, tag="emb")
                nc.vector.tensor_scalar_mul(emb[:m], acc[:m, 0:NOUT], rs[:m])
                emb2 = sp.tile([128, NOUT], F32, tag="emb2")
                nc.vector.tensor_tensor(emb2[:m], emb[:m], b2_sb[:m], op=OP.add)
                nc.sync.dma_start(emb_out[w * 128:w * 128 + m, :], emb2[:m])
                sq = sp.tile([128, NOUT], F32, tag="sq")
                nc.vector.tensor_tensor(sq[:m], emb2[:m], emb2[:m], op=OP.mult)
                ssq = sp.tile([128, 2], F32, tag="ssq")
                nc.vector.tensor_reduce(ssq[:m, 0:1], sq[:m], axis=AX.X,
                                        op=OP.add)
                nc.scalar.activation(ssq[:m, 1:2], ssq[:m, 0:1], AF.Sqrt)
                rn = sp.tile([128, 1], F32, tag="rn")
                nc.vector.reciprocal(rn[:m], ssq[:m, 1:2])
                nc.vector.tensor_scalar_mul(dnx_sb[:m, w, 0:NOUT], emb2[:m],
                                            rn[:m])

            edge_phase(hx2_full.rearrange("(n f) -> n f", f=ROW2), ROW2,
                       1, NOUT + 1, ad2_sb, epi2)

            # ================= phase 5: soft k-means =================
            with tc.tile_pool(name="km", bufs=1) as kp, \
                 tc.tile_pool(name="kmu", bufs=2) as kmu, \
                 tc.tile_pool(name="kw", bufs=2) as kw, \
                 tc.tile_pool(name="kps", bufs=1, space="PSUM") as kps, \
                 tc.tile_pool(name="ktp", bufs=1, space="PSUM") as ktp, \
                 tc.tile_pool(name="kp1", bufs=1, space="PSUM") as kp1:
                dnT = kp.tile([64, NPC], F32, tag="dnT")
                for nb in range(NB):
                    m = blk(nb)
                    tps = ktp.tile([64, 128], F32, space="PSUM", tag="dntp")
                    nc.tensor.transpose(tps[:NOUT, :m], dnx_sb[:m, nb, 0:NOUT],
                                        ident[:m, :m])
                    nc.vector.tensor_copy(dnT[:, nb * 128:nb * 128 + m],
                                          tps[:NOUT, :m])
                muT = kmu.tile([64, K], F32, tag="muT")
                nc.sync.dma_start(muT[:], mu0_in[:])
                mu_f = None
                for it in range(niter + 2):
                    last = it == niter + 1
                    dps = [kps.tile([128, 5 * K], F32, space="PSUM", name=f"dps{b}",
                                    tag=f"dps{b}") for b in range(4)]
                    for nb in range(NB):
                        m = blk(nb)
                        nc.tensor.matmul(
                            dps[nb // 5][:m, (nb % 5) * K:(nb % 5 + 1) * K],
                            lhsT=dnT[:, nb * 128:nb * 128 + m], rhs=muT[:],
                            start=True, stop=True, skip_group_check=True)
                    ext = kw.tile([128, NB * K], F32, tag="ext")
                    for b in range(4):
                        nc.scalar.activation(ext[:, b * 5 * K:(b + 1) * 5 * K],
                                             dps[b][:], AF.Exp, scale=TEMP)
                    rsum = kw.tile([128, NB], F32, tag="rsum")
                    nc.vector.tensor_reduce(
                        rsum[:], ext[:].rearrange("p (w k) -> p w k", k=K),
                        axis=AX.X, op=OP.add)
                    rn = kw.tile([128, NB], F32, tag="rn")
                    nc.vector.reciprocal(rn[:], rsum[:])
                    rt = kw.tile([128, NB * K], F32, tag="rt")
                    for nb in range(NB):
                        nc.vector.tensor_scalar_mul(
                            rt[:, nb * K:(nb + 1) * K],
                            ext[:, nb * K:(nb + 1) * K], rn[:, nb:nb + 1])
                    if last:
                        dist_f = kw.tile([128, NB * K], F32, tag="dist_f")
                        for b in range(4):
                            nc.vector.tensor_copy(
                                dist_f[:, b * 5 * K:(b + 1) * 5 * K], dps[b][:])
                        for nb in range(NB):
                            m = blk(nb)
                            nc.sync.dma_start(
                                dist_out[nb * 128:nb * 128 + m, :],
                                dist_f[:m, nb * K:(nb + 1) * K])
                            nc.sync.dma_start(
                                r_out[nb * 128:nb * 128 + m, :],
                                rt[:m, nb * K:(nb + 1) * K])
                        nc.sync.dma_start(mu_out[:], mu_f[:])
                        break
                    pacc = kp1.tile([K, NOUT + 1], F32, space="PSUM", tag="pacc")
                    for nb in range(NB):
                        m = blk(nb)
                        nc.tensor.matmul(
                            pacc[:], lhsT=rt[:m, nb * K:(nb + 1) * K],
                            rhs=dnx_sb[:m, nb, :], start=(nb == 0),
                            stop=(nb == NB - 1), skip_group_check=True)
                    pl = kw.tile([K, NOUT + 1], F32, tag="pl")
                    nc.vector.tensor_copy(pl[:], pacc[:])
                    nc.sync.dma_start(part_loc[:], pl[:])
                    nc.gpsimd.collective_compute(
                        "AllGather", OP.bypass, replica_groups=rg,
                        ins=[part_loc.rearrange("k f -> (k f)")],
                        outs=[part_full[:]])
                    psb = kw.tile([K, NCORES, NOUT + 1], F32, tag="psb")
                    nc.sync.dma_start(
                        psb[:], part_full.rearrange("(r k f) -> k r f", k=K,
                                                    f=NOUT + 1))
                    cm = kw.tile([K, NOUT + 1], F32, tag="cm")
                    nc.vector.tensor_reduce(
                        cm[:], psb[:].rearrange("k r f -> k f r"),
                        axis=AX.X, op=OP.add)
                    crr = kw.tile([K, 1], F32, tag="crr")
                    nc.vector.reciprocal(crr[:], cm[:, NOUT:NOUT + 1])
                    mu_f = kmu.tile([K, NOUT], F32, tag="mu_f")
                    nc.vector.tensor_scalar_mul(mu_f[:], cm[:, 0:NOUT], crr[:])
                    mtp = ktp.tile([64, K], F32, space="PSUM", tag="mtp")
                    nc.tensor.transpose(mtp[:NOUT, :], mu_f[:], ident[:K, :K])
                    muT = kmu.tile([64, K], F32, tag="muT")
                    nc.vector.tensor_copy(muT[:], mtp[:NOUT, :])
    nc.compile()
    return nc


def _prep_inputs(x, W1, a_src1, a_dst1, b1, W2, a_src2, a_dst2, b2, mu0):
    """Float-side host prep: transposes + folded attention projections."""
    x = np.ascontiguousarray(np.asarray(x, np.float32))
    W1T = np.asarray(W1, np.float32).T.copy()
    a_src1 = np.asarray(a_src1, np.float32)
    a_dst1 = np.asarray(a_dst1, np.float32)
    wa_s1 = np.stack([W1T[:, h * NHID:(h + 1) * NHID] @ a_src1[h]
                      for h in range(HEADS)], axis=1)
    wa_d1 = np.stack([W1T[:, h * NHID:(h + 1) * NHID] @ a_dst1[h]
                      for h in range(HEADS)], axis=1)
    w1_ext = np.ascontiguousarray(
        np.concatenate([W1T, wa_s1, wa_d1], axis=1))              # [512, 260]
    W2T = np.asarray(W2, np.float32).T.copy()
    wa_s2 = (W2T @ np.asarray(a_src2, np.float32)[0])[:, None]
    wa_d2 = (W2T @ np.asarray(a_dst2, np.float32)[0])[:, None]
    w2_ext = np.ascontiguousarray(
        np.concatenate([W2T, wa_s2, wa_d2], axis=1))              # [256, 66]
    b1rep = np.ascontiguousarray(
        np.tile(np.asarray(b1, np.float32)[None, :], (128, 1)))
    b2rep = np.ascontiguousarray(
        np.tile(np.asarray(b2, np.float32)[None, :] + np.float32(1e-6),
                (128, 1)))
    mu0T = np.ascontiguousarray(np.asarray(mu0, np.float32).T)    # [64, 100]
    xT = np.ascontiguousarray(x.T)                                # [512, 20000]
    return xT, w1_ext, w2_ext, b1rep, b2rep, mu0T


def kernel(x, W1, a_src1, a_dst1, b1, W2, a_src2, a_dst2, b2, mu0,
           edge_index, num_iter):
    edge_index = np.asarray(edge_index).astype(np.int64)
    niter = int(np.asarray(num_iter))
    cmax, idx16, oh, ohT = _host_prep(edge_index)
    xT, w1_ext, w2_ext, b1rep, b2rep, mu0T = _prep_inputs(
        x, W1, a_src1, a_dst1, b1, W2, a_src2, a_dst2, b2, mu0)

    key = (cmax, niter)
    if key not in _cache:
        _cache[key] = _build(cmax, niter)
    nc = _cache[key]

    in_maps = []
    for c in range(NCORES):
        in_maps.append({
            "xT_in": np.ascontiguousarray(xT[:, c * NPC:(c + 1) * NPC]),
            "w1_in": w1_ext, "w2_in": w2_ext,
            "b1_in": b1rep, "b2_in": b2rep, "mu0_in": mu0T,
            "idx_in": idx16[c], "oh_in": oh[c], "ohT_in": ohT[c],
        })
    res = run_bass_kernel_spmd(nc, in_maps, core_ids=list(range(NCORES)),
                               trace=TRACE)
    global LAST_RESULT
    LAST_RESULT = res
    mu = res.results[0]["mu_out"]
    r = np.concatenate([res.results[c]["r_out"] for c in range(NCORES)], axis=0)
    emb = np.concatenate([res.results[c]["emb_out"] for c in range(NCORES)],
                         axis=0)
    dist = np.concatenate([res.results[c]["dist_out"] for c in range(NCORES)],
                          axis=0)
    return mu, r, emb, dist
